# revision 32
# baseline (speedup 1.0000x reference)
"""Multi-head causal attention (B=4, L=2048, E=1024, H=16) on 8 trn2 NeuronCores.

Sharding: (batch, head-group) grid - core c handles batch b=c//2 and heads
g=c%2 (8 heads each).  Each core computes its heads' QKV projection, causal
attention, and a partial output projection; the host sums the two partials
per batch.

v2 design (vs the A0/A1-halves baseline):
  - lq is processed in 4 chunks of 512 so softmax exp (the serial ACT-engine
    bottleneck, ~(N+352)/1.2 ns per instruction) starts ~8us into the kernel
    and stays saturated to the end.
  - scores for each head pair run CONCURRENTLY on the PE via row tiling:
    head A occupies array rows 0:63 (tile_position (0,0)), head B rows
    64:127 ((64,0)), each writing its own PSUM bank -> scores stream cost
    halves and the array is fully occupied (HAM-friendly).
  - everything is bf16 into the PE (f32 PSUM accumulation): same PE rate as
    float32r but half the SBUF/DMA traffic; expected rel err ~5e-3 vs the
    2e-2 gate.
  - weight/x DMAs are split into need-ordered granules so the first matmul
    issues at ~3us instead of ~33us.
  - dense work (qkv projection of later chunks, deferred output projection)
    is kept in a queue and pumped between attention t-steps to fill PE idle
    time and keep the HAM clock gate at 2.4GHz; outproj of chunks 0..2 is
    deferred into the last attention chunks which have no other dense work.
"""

import numpy as np

L = 2048
E = 1024
NH = 8        # heads per core
D = 64
JQ = 512      # feature rows per core (NH*D)
ET = E // 128  # 8 e-tiles
LT = L // 128  # 16 l-tiles
NC = 4        # lq chunks
CW = 512      # chunk width

_CACHE = {}


def build_nc():
    import concourse.mybir as mybir
    import concourse.tile as tile
    from concourse import bacc
    from contextlib import ExitStack

    f32 = mybir.dt.float32
    bf16 = mybir.dt.bfloat16
    Exp = mybir.ActivationFunctionType.Exp

    nc = bacc.Bacc("TRN2", target_bir_lowering=False, debug=False)

    # host pre-arranged so every DMA granule is contiguous on both sides
    xh_d = nc.declare_dram_parameter("xh", [NC, ET, 128, CW], bf16, isOutput=False)
    wqkh_d = nc.declare_dram_parameter("wqkh", [4, ET, 128, 256], bf16, isOutput=False)
    wvh_d = nc.declare_dram_parameter("wvh", [ET, 128, JQ], bf16, isOutput=False)
    woh_d = nc.declare_dram_parameter("woh", [4, 128, E], bf16, isOutput=False)
    diag2_d = nc.declare_dram_parameter("diag2", [128, 2, 128], bf16, isOutput=False)
    y_d = nc.declare_dram_parameter("y", [L, E], f32, isOutput=True)

    with ExitStack() as ctx:
        tc = ctx.enter_context(tile.TileContext(nc))

        consts = ctx.enter_context(tc.tile_pool(name="consts", bufs=1))
        diag2_sb = consts.tile([128, 2, 128], bf16)

        # persistent attention operands (bf16)
        qkv_p = ctx.enter_context(tc.tile_pool(name="qkv", bufs=1))
        v_aug = qkv_p.tile([128, LT, NH, 65], bf16)
        qT_sb = qkv_p.tile([128, 4, L], bf16)
        kT_sb = qkv_p.tile([128, 4, L], bf16)
        aoT_sb = qkv_p.tile([128, 4, L], bf16)

        w_p = ctx.enter_context(tc.tile_pool(name="w", bufs=1))
        wqkT_sb = w_p.tile([128, ET, 2 * JQ], bf16)
        wvT_sb = w_p.tile([128, ET, JQ], bf16)
        woT_sb = w_p.tile([128, 4, E], bf16)

        xc_p = ctx.enter_context(tc.tile_pool(name="xc", bufs=3))
        pe_p = ctx.enter_context(tc.tile_pool(name="pe", bufs=3))
        aou_p = ctx.enter_context(tc.tile_pool(name="aou", bufs=2))
        rc_p = ctx.enter_context(tc.tile_pool(name="rc", bufs=2))
        rcb_p = ctx.enter_context(tc.tile_pool(name="rcb", bufs=2))
        rcd_p = ctx.enter_context(tc.tile_pool(name="rcd", bufs=2, space="DRAM"))
        y_p = ctx.enter_context(tc.tile_pool(name="y", bufs=3))
        ypart_p = ctx.enter_context(tc.tile_pool(name="ypart", bufs=8))

        scp_p = ctx.enter_context(tc.tile_pool(name="scp", bufs=2, space="PSUM"))
        pva_p = ctx.enter_context(tc.tile_pool(name="pva", bufs=1, space="PSUM"))
        pvb_p = ctx.enter_context(tc.tile_pool(name="pvb", bufs=1, space="PSUM"))
        dps_p = ctx.enter_context(tc.tile_pool(name="dps", bufs=2, space="PSUM"))

        # ---------------- startup DMA granules, need-ordered ----------------
        nc.vector.memset(v_aug[:, :, :, 64:65], 1.0)
        nc.sync.dma_start(out=diag2_sb, in_=diag2_d.ap())
        # need-ordered granules: a tiny first slice lets the et=0 matmuls
        # start while the rest streams
        for lo, hi in ((0, 1), (1, 4), (4, 8)):
            nc.sync.dma_start(
                out=wvT_sb[:, lo:hi, :],
                in_=wvh_d.ap()[lo:hi].rearrange("et p j -> p et j"),
            )
        xc_tiles = {}

        def load_xc(c):
            if c not in xc_tiles:
                xc_tiles[c] = xc_p.tile([128, ET, CW], bf16, tag="xc", name=f"xc{c}")
            grans = ((0, 1), (1, 4), (4, 8)) if c == 0 else ((0, 4), (4, 8))
            for lo, hi in grans:
                nc.sync.dma_start(
                    out=xc_tiles[c][:, lo:hi, :],
                    in_=xh_d.ap()[c, lo:hi].rearrange("et p w -> p et w"),
                )

        def load_wqk(b, cols):
            nc.sync.dma_start(
                out=wqkT_sb[:, :, cols[0]:cols[1]],
                in_=wqkh_d.ap()[b].rearrange("et p j -> p et j"),
            )

        load_xc(0)
        load_wqk(0, (0, 256))      # q for pairs 0, 1
        load_wqk(1, (512, 768))    # k for pairs 0, 1
        load_wqk(2, (256, 512))    # q pairs 2, 3
        load_wqk(3, (768, 1024))   # k pairs 2, 3

        # ---------------- dense units ----------------
        def v_unit(lt):
            xc = xc_tiles[lt // 4]
            i = lt % 4
            ps = dps_p.tile([128, JQ], f32, tag="dps")
            for et in range(ET):
                nc.tensor.matmul(
                    ps,
                    lhsT=xc[:, et, i * 128:(i + 1) * 128],
                    rhs=wvT_sb[:, et, :],
                    start=(et == 0), stop=(et == ET - 1),
                )
            nc.vector.tensor_copy(
                out=v_aug[:, lt, :, 0:64],
                in_=ps.rearrange("p (h d) -> p h d", h=NH),
            )

        def qk_unit(jt, c):
            xc = xc_tiles[c]
            ps = dps_p.tile([128, CW], f32, tag="dps", name="qkps")
            dst = qT_sb if jt < 4 else kT_sb
            for et in range(ET):
                nc.tensor.matmul(
                    ps,
                    lhsT=wqkT_sb[:, et, jt * 128:(jt + 1) * 128],
                    rhs=xc[:, et, :],
                    start=(et == 0), stop=(et == ET - 1),
                )
            nc.vector.tensor_copy(out=dst[:, jt % 4, c * CW:(c + 1) * CW], in_=ps)

        def op_unit(lt, ec):
            ps = dps_p.tile([128, CW], f32, tag="dps", name="opps")
            for jt in range(4):
                nc.tensor.matmul(
                    ps,
                    lhsT=aoT_sb[:, jt, lt * 128:(lt + 1) * 128],
                    rhs=woT_sb[:, jt, ec * CW:(ec + 1) * CW],
                    start=(jt == 0), stop=(jt == 3),
                )
            yt = y_p.tile([128, CW], f32, tag="y")
            nc.vector.tensor_copy(out=yt, in_=ps)
            nc.sync.dma_start(
                out=y_d.ap()[lt * 128:(lt + 1) * 128, ec * CW:(ec + 1) * CW],
                in_=yt,
            )

        # last-chunk outproj is split so only the jt=3 (last head pair)
        # contribution remains after the final normalize
        yparts = {}

        def op_partial(lt, ec):
            ps = dps_p.tile([128, CW], f32, tag="dps", name="oppart")
            for jt in range(3):
                nc.tensor.matmul(
                    ps,
                    lhsT=aoT_sb[:, jt, lt * 128:(lt + 1) * 128],
                    rhs=woT_sb[:, jt, ec * CW:(ec + 1) * CW],
                    start=(jt == 0), stop=(jt == 2),
                )
            yp = ypart_p.tile([128, CW], f32, tag="ypart")
            nc.vector.tensor_copy(out=yp, in_=ps)
            yparts[(lt, ec)] = yp

        def op_finish(lt, ec):
            ps = dps_p.tile([128, CW], f32, tag="dps", name="opfin")
            nc.tensor.matmul(
                ps,
                lhsT=aoT_sb[:, 3, lt * 128:(lt + 1) * 128],
                rhs=woT_sb[:, 3, ec * CW:(ec + 1) * CW],
                start=True, stop=True,
            )
            yt = y_p.tile([128, CW], f32, tag="y")
            nc.vector.tensor_add(out=yt, in0=yparts[(lt, ec)], in1=ps)
            nc.sync.dma_start(
                out=y_d.ap()[lt * 128:(lt + 1) * 128, ec * CW:(ec + 1) * CW],
                in_=yt,
            )

        # ---------------- dense queue (emission-order = PE execution order).
        # attention() force-emits its own data dependencies via need(); the
        # cadenced pump() just drains the rest to fill PE idle time.
        queue = []
        for c in (1, 2, 3):
            queue.append((("qk", 0, c), lambda jt=0, c=c: qk_unit(jt, c)))
            queue.append((("qk", 4, c), lambda jt=4, c=c: qk_unit(jt, c)))
            for i in range(4):
                queue.append((("v", 4 * c + i), lambda lt=4 * c + i: v_unit(lt)))
            for p in (1, 2, 3):
                queue.append((("qk", p, c), lambda jt=p, c=c: qk_unit(jt, c)))
                queue.append((("qk", 4 + p, c), lambda jt=4 + p, c=c: qk_unit(jt, c)))
        for c in (0, 1, 2):
            for lt in range(4 * c, 4 * c + 4):
                for ec in range(2):
                    queue.append((("op", lt, ec), lambda lt=lt, ec=ec: op_unit(lt, ec)))

        def pump(n=1):
            for _ in range(min(n, len(queue))):
                queue.pop(0)[1]()

        def need(key):
            for i, (k, fn) in enumerate(queue):
                if k == key:
                    queue.pop(i)
                    fn()
                    return

        # ---------------- attention ----------------
        def attention(p, c):
            nt = 4 * (c + 1)
            stride = (99, 2, 2, 3)[c]  # t-pump cadence per chunk
            need(("qk", p, c))
            need(("qk", 4 + p, c))
            pva = pva_p.tile([65, CW], f32, tag="pva")
            pvb = pvb_p.tile([65, CW], f32, tag="pvb")

            def emit_pv(pend, stop):
                pe, t, off = pend
                for j, pv in ((0, pva), (1, pvb)):
                    nc.tensor.matmul(
                        pv[:, off:CW],
                        lhsT=v_aug[:, t, 2 * p + j, :],
                        rhs=pe[:, j, off:CW],
                        start=(t == 0), stop=stop,
                        skip_group_check=True,
                    )

            pending = None
            for t in range(nt):
                off = max(0, 128 * t - CW * c)
                scp = scp_p.tile([128, 2, CW], f32, tag="scp")
                for j, po in ((0, 0), (1, 64)):
                    nc.tensor.matmul(
                        scp[:, j, off:CW],
                        lhsT=kT_sb[po:po + 64, p, t * 128:(t + 1) * 128],
                        rhs=qT_sb[po:po + 64, p, CW * c + off:CW * (c + 1)],
                        start=True, stop=True,
                    )
                pe = pe_p.tile([128, 2, CW], bf16, tag="pe")
                nc.scalar.activation(
                    out=pe[:, :, off:CW], in_=scp[:, :, off:CW],
                    func=Exp, scale=0.125,
                )
                if 128 * t >= CW * c:  # diagonal tile: zero lk > lq
                    nc.vector.tensor_mul(
                        out=pe[:, :, off:off + 128],
                        in0=pe[:, :, off:off + 128],
                        in1=diag2_sb,
                    )
                if pending is not None:
                    need(("v", pending[1]))
                    emit_pv(pending, stop=False)
                pending = (pe, t, off)
                if t % stride == stride - 1:
                    pump(1)
            need(("v", pending[1]))
            emit_pv(pending, stop=True)
            return pva, pvb

        def normalize(p, c, pva, pvb):
            # softmax sums live on partition 64 (ones column of v_aug):
            # respread to 128 lanes via DRAM for the reciprocal, then
            # broadcast back (stride-0 partition reads need DRAM).  The
            # chain reads PSUM directly so it runs parallel to the copies.
            aoU = aou_p.tile([65, 2, CW], f32, tag="aou")
            nc.vector.tensor_copy(out=aoU[:, 0, :], in_=pva)
            nc.vector.tensor_copy(out=aoU[:, 1, :], in_=pvb)
            rcd = rcd_p.tile([1, 1024], f32, tag="rcd")
            nc.sync.dma_start(
                out=rcd, in_=aoU[64:65, :, :].rearrange("o j d -> o (j d)"))
            rc8 = rc_p.tile([128, 8], f32, tag="rc8")
            nc.sync.dma_start(out=rc8, in_=rcd.rearrange("o (p x) -> (o p) x", p=128))
            nc.vector.reciprocal(out=rc8, in_=rc8)
            rcd2 = rcd_p.tile([1, 1024], f32, tag="rcd2")
            nc.sync.dma_start(out=rcd2.rearrange("o (p x) -> (o p) x", p=128), in_=rc8)
            rcb = rcb_p.tile([64, 1024], f32, tag="rcb")
            nc.sync.dma_start(out=rcb, in_=rcd2.to_broadcast((64, 1024)))
            cs = slice(CW * c, CW * (c + 1))
            nc.vector.tensor_mul(
                out=aoT_sb[0:64, p, cs], in0=aoU[0:64, 0, :], in1=rcb[:, 0:CW])
            nc.vector.tensor_mul(
                out=aoT_sb[64:128, p, cs], in0=aoU[0:64, 1, :], in1=rcb[:, CW:2 * CW])

        # ---------------- main schedule ----------------
        for lt in (3, 2, 1):  # pulled by attention(0,0)'s per-tile needs
            queue.insert(0, (("v", lt), lambda lt=lt: v_unit(lt)))
        v_unit(0)
        qk_unit(0, 0)
        qk_unit(4, 0)
        for c in range(NC):
            for p in range(4):
                if c == 0 and p < 3:  # just-in-time q/k for the next pair
                    qk_unit(p + 1, 0)
                    qk_unit(4 + p + 1, 0)
                pva, pvb = attention(p, c)
                normalize(p, c, pva, pvb)
                if c == 0 and p == 0:
                    load_xc(1)
                if c == 0 and p == 2:
                    load_xc(2)
                if c == 1 and p == 2:
                    load_xc(3)
                if p == 1 and c == 0:
                    for jt in range(4):
                        nc.sync.dma_start(
                            out=woT_sb[:, jt, :], in_=woh_d.ap()[jt])
                if c > 0 or p >= 2:
                    pump(1)
        # the jt<=2 outproj partials of the last chunk execute during the
        # final reciprocal-chain wait; heaters bridge any remaining idle so
        # the jt=3 finishes run HAM-warm
        for lt in range(12, 16):
            for ec in range(2):
                op_partial(lt, ec)
        pump(len(queue))  # flush leftovers
        for i in range(8):
            hps = dps_p.tile([128, CW], f32, tag="dps", name="heat")
            nc.tensor.matmul(
                hps, lhsT=qT_sb[:, i % 4, 0:128], rhs=qT_sb[:, i % 4, 0:CW],
                start=True, stop=True,
            )
        for lt in range(12, 16):
            for ec in range(2):
                op_finish(lt, ec)

    nc.compile()
    return nc


def make_in_maps(x, w_qkv, wo):
    """Host-side sharding: 8 cores = (batch b=c//2, head-group g=c%2)."""
    import ml_dtypes
    bf16 = ml_dtypes.bfloat16
    x = np.asarray(x, dtype=np.float32)
    w_qkv = np.asarray(w_qkv, dtype=np.float32)
    wo = np.asarray(wo, dtype=np.float32)
    diag = np.triu(np.ones((128, 128), np.float32))
    diag2 = np.ascontiguousarray(
        np.stack([diag, diag], axis=1)).astype(bf16)  # [128, 2, 128]
    in_maps = []
    for c in range(8):
        b, g = c // 2, c % 2
        js = slice(g * JQ, (g + 1) * JQ)
        wq = w_qkv[0:E][js]
        wk = w_qkv[E:2 * E][js]
        wv = w_qkv[2 * E:3 * E][js]
        xT = x[b].T  # [E, L]
        # granule-contiguous layouts: [.., et, 128, cols]
        xh = xT.reshape(ET, 128, NC, CW).transpose(2, 0, 1, 3)
        wqkT = np.concatenate([wq, wk], 0).T  # [E, 1024]
        wqkh = np.stack([
            wqkT[:, c0:c1].reshape(ET, 128, 256)
            for (c0, c1) in ((0, 256), (512, 768), (256, 512), (768, 1024))
        ])
        in_maps.append({
            "xh": np.ascontiguousarray(xh).astype(bf16),
            "wqkh": np.ascontiguousarray(wqkh).astype(bf16),
            "wvh": np.ascontiguousarray(wv.T.reshape(ET, 128, JQ)).astype(bf16),
            "woh": np.ascontiguousarray(wo[:, js].T.reshape(4, 128, E)).astype(bf16),
            "diag2": diag2,
        })
    return in_maps


def _get_nc():
    if "nc" not in _CACHE:
        _CACHE["nc"] = build_nc()
    return _CACHE["nc"]


def kernel(x, mask, w_qkv, wo, _trace=False, _trace_kwargs=None):
    from concourse.bass_utils import run_bass_kernel_spmd

    nc = _get_nc()
    in_maps = make_in_maps(x, w_qkv, wo)
    res = run_bass_kernel_spmd(
        nc, in_maps, core_ids=list(range(8)),
        trace=_trace, **(_trace_kwargs or {}),
    )
    _CACHE["last_results"] = res
    y = np.stack([res.results[2 * b]["y"] + res.results[2 * b + 1]["y"] for b in range(4)])
    return y.astype(np.float32)
